# revision 23
# baseline (speedup 1.0000x reference)
"""Trainium2 Bass kernel for nn_Net_39230231281866 (dense_cnn).

Network: conv3x3(1->6) -> Taylor-sigmoid -> conv3x3(6->7) -> flatten
         -> fc(4032->128) -> sigmoid -> fc(128->10) -> log_softmax,
batch 8192, data-parallel over 8 NeuronCores (1024 samples/core).

Mapping:
  * conv2+fc1 folded on the host into one dense GEMM W_comb [128, 4056].
  * conv1 = banded-weight matmul (K = input-pixel window, M <= 128 output
    positions x channels, batch on the moving free dim), fp16, tiled as
    34 dense chunks (24x(3,7) + 6x(4,5) + 3x(2,8) + 1x(2,2)).
  * The folded GEMM runs in fp8e4m3 with MatmulPerfMode.DoubleRow: chunk
    pairs are contracted 256-deep per instruction, halving the second
    GEMM's matmul count. W_comb is scaled by a power of two (Sw) into
    fp8 range; the inverse scale rides the fc1-sigmoid exp's free affine.
  * Taylor-sigmoid: custom DVE op computes den16(u) = u^4+2u^3+3u^2+3u+3
    where u = (-conv1(x)-b1)/2 (scale folded into the conv weights); the
    reciprocal runs on ScalarE (Reciprocal PWP table) as s8 = 16/den16,
    written directly as fp8e4m3 (range ~(0, 8] keeps fp8 normals).
  * fc1 sigmoid = exp on ScalarE (natural_log_exp table set, shared with
    the log_softmax tail -> one table switch total) + 1/(1+e) on the DVE
    via RECIPROCAL_APPROX_FAST.
  * conv1 outputs land in PSUM-bank pairs (triple-buffered, 6 of 8 banks)
    so elementwise ops run at free-dim 1024; the z accumulator and the
    fc2 psum share the remaining 2 banks.
  * The PE stream is software-pipelined: group g's DoubleRow matmuls are
    emitted after group g+1's conv matmuls, hiding the taylor+reciprocal
    latency so the PE never stalls on the s-tile.
  * dma_start issue costs ~0.8us on an engine queue, so window-DMA issue
    alternates between the Sync and (otherwise idle) GpSimd queues; the
    big fp8 wcpack load is deferred until after the first window DMAs and
    its pre-observe dummy matmul is emitted after the first conv group.
"""

import os
import numpy as np
import ml_dtypes

_B = 8192
_NCORES = 8
_PC = _B // _NCORES
_SLICE = 512
_NSL = _PC // _SLICE

LAST_RESULTS = None


def _tiles():
    """conv1 output tiling, 34 tiles (vs naive 36): 24x(3,7) M=126 +
    6x(4,5) M=120 + 3x(2,8) M=96 + 1x(2,2) M=24 -- denser M packing cuts
    PE/DVE/ACT work ~6%. Sorted by class so psum-pair mates share a bias
    vector; the final pair is mixed-class (handled as two half DVE ops)."""
    specs = []
    for oy0 in range(0, 24, 3):
        for ox0 in (0, 7, 14):
            specs.append((oy0, 3, ox0, 7))
    for oy0 in range(0, 24, 4):
        specs.append((oy0, 4, 21, 5))
    for ox0 in (0, 8, 16):
        specs.append((24, 2, ox0, 8))
    specs.append((24, 2, 24, 2))
    ts = []
    for (oy0, noy, ox0, nox) in specs:
        ts.append(dict(oy0=oy0, noy=noy, ox0=ox0, nox=nox,
                       ky=noy + 2, kx=nox + 2,
                       K=(noy + 2) * (nox + 2), M=noy * nox * 6,
                       cls=(noy, nox)))
    order = {(3, 7): 0, (4, 5): 1, (2, 8): 2, (2, 2): 3}
    ts.sort(key=lambda t: order[t["cls"]])
    return ts


def _host_prep(x, w1, b1, w2, b2, fw1, fb1, fw2, fb2):
    x = np.asarray(x, np.float32)
    w1 = np.asarray(w1, np.float32); b1 = np.asarray(b1, np.float32)
    w2 = np.asarray(w2, np.float32); b2 = np.asarray(b2, np.float32)
    fw1 = np.asarray(fw1, np.float32); fb1 = np.asarray(fb1, np.float32)
    fw2 = np.asarray(fw2, np.float32); fb2 = np.asarray(fb2, np.float32)

    tiles = _tiles()

    # banded conv1 weights, scaled by -1/2 (u = (-conv-b1)/2), M padded to 128
    cls_list = [(3, 7), (4, 5), (2, 8), (2, 2)]
    cls_idx = {c: i for i, c in enumerate(cls_list)}
    w1pack = np.zeros((45, 128 * 4), np.float32)
    biaspack = np.zeros((128, 4), np.float32)
    for cls in cls_list:
        noy, nox = cls
        kx = nox + 2
        ci = cls_idx[cls]
        for oy in range(noy):
            for ox in range(nox):
                for oc in range(6):
                    m = (oy * nox + ox) * 6 + oc
                    biaspack[m, ci] = -0.5 * b1[oc]
                    for dy in range(3):
                        for dx in range(3):
                            k = (oy + dy) * kx + (ox + dx)
                            w1pack[k, 128 * ci + m] = -0.5 * w1[oc, 0, dy, dx]

    # fold conv2 + fc1 -> W_comb [128, 6*26*26], b_comb. The reference's
    # 1.5 factor (s = 1.5/den16) moves into the z unscale: the kernel
    # computes s8 = 16/den16, so z_true = z_psum * 1.5/(16*Sw).
    fw1r = fw1.reshape(128, 7, 24, 24)
    Wc = np.zeros((128, 6, 26, 26), np.float32)
    for dy in range(3):
        for dx in range(3):
            Wc[:, :, dy:dy + 24, dx:dx + 24] += np.einsum(
                "joyx,oi->jiyx", fw1r, w2[:, :, dy, dx], optimize=True)
    b_comb = fb1 + np.einsum("joyx,o->j", fw1r, b2)
    Wc_flat = Wc.reshape(128, 6 * 26 * 26)

    # fp8 scale: power of two that puts max|Wc| around 240 (e4m3 max 448)
    Sw = float(2.0 ** np.floor(np.log2(240.0 / np.abs(Wc_flat).max())))

    # W_comb columns in conv1-chunk partition order, packed [128, 36*128] fp8
    f8 = ml_dtypes.float8_e4m3fn
    wcpack = np.zeros((128, 128 * len(tiles)), np.float32)
    for t_i, t in enumerate(tiles):
        rows = []
        for oy in range(t["noy"]):
            for ox in range(t["nox"]):
                for oc in range(6):
                    rows.append((oc * 26 + t["oy0"] + oy) * 26 + t["ox0"] + ox)
        wcpack[:t["M"], 128 * t_i:128 * t_i + 128] = (Sw * Wc_flat)[:, rows].T

    f16 = np.float16
    # one f32 const blob: [0:4] taylor bias per class, [4:5] -b_comb,
    # [5:45] fb2 tiled x4 -- single DMA instead of three
    cst32 = np.zeros((128, 45), np.float32)
    cst32[:, 0:4] = biaspack
    cst32[:, 4] = -b_comb
    cst32[:, 5:45] = np.tile(fb2.reshape(1, 10), (128, 4))
    consts = dict(
        wcpack=wcpack.astype(f8), w1pack=w1pack.astype(f16),
        cst32=cst32, cls_idx=cls_idx, Sw=Sw,
        fw2t=np.ascontiguousarray(fw2.T).astype(f16),                   # [128, 10]
    )
    # pre-windowed input: for each conv tile a contiguous [K_t, B] block of
    # pixel-major rows, so each per-slice window DMA is one dense 2D transfer
    x_pm = x.reshape(_B, 784).T.astype(f16)                             # [784, B]
    wins = []
    for t in tiles:
        rows = (np.arange(t["ky"])[:, None] + t["oy0"]) * 28 + \
               (np.arange(t["kx"])[None, :] + t["ox0"])
        wins.append(x_pm[rows.reshape(-1), :])                          # [K_t, B]
    consts["win_offs"] = np.cumsum([0] + [2 * t["K"] for t in tiles])
    return wins, consts, tiles


def _register_taylor_den16():
    import concourse.dve_ops as dve_ops
    if "TAYLOR_DEN16_ANT" in dve_ops._SUB_OPCODE_FOR_NAME:
        return next(o for o in dve_ops.OPS if o.name == "TAYLOR_DEN16_ANT")
    from concourse.dve_spec import Spec, Src0, C0, C1, C2

    # u = in0 + s0;  out = u^4 + 2u^3 + 3u^2 + 3u + 3  ==  (q(t)+48)/16
    u = Src0 + C0
    body = ((((u + C1) * u + C2) * u + C2) * u + C2)

    def _ref(in0, in1, s0, s1, imm2):
        xx = in0.astype(np.float32) + s0
        return (((xx + s1) * xx + imm2) * xx + imm2) * xx + imm2

    op = dve_ops.DveOp(
        "TAYLOR_DEN16_ANT",
        Spec(body=body, reference=_ref),
        subdim=False,
        uops_sha={"v3": "0d84493259836d20", "v4": "be052b2c26b42830"},
    )
    dve_ops.OPS.append(op)
    dve_ops.CUSTOM_DVE_SPECS[op.name] = op.spec
    row = max(dve_ops._SUB_OPCODE_FOR_NAME.values()) + 1
    assert row < 0x20
    dve_ops._SUB_OPCODE_FOR_NAME[op.name] = row
    return op


def _pin_exp_ln_table():
    """Make Exp and Ln resolve only to natural_log_exp_and_others so the
    whole post-reciprocal tail costs one table load."""
    import concourse.bacc as bacc
    import concourse.mybir as mybir
    if getattr(bacc, "_ant_expln_pinned", False):
        return
    orig = bacc.get_activation_tables
    AF = mybir.ActivationFunctionType

    def patched(arch):
        tabs = {k: set(v) for k, v in orig(arch).items()}
        for name, fns in tabs.items():
            if name != "natural_log_exp_and_others":
                fns.discard(AF.Exp)
                fns.discard(AF.Ln)
        return tabs

    bacc.get_activation_tables = patched
    bacc._ant_expln_pinned = True


def _act_raw(nc, out, in_, func, bias=0.0, scale=1.0):
    """Emit InstActivation directly (used for Reciprocal, which the
    nc.scalar.activation wrapper refuses; measured ~1.2e-5 rel err)."""
    import concourse.mybir as mybir
    eng = nc.scalar
    inputs = [eng.lower_ap(in_)]
    for arg in (bias, scale, 0.0):
        inputs.append(mybir.ImmediateValue(dtype=mybir.dt.float32,
                                           value=float(arg)))
    return eng.add_instruction(mybir.InstActivation(
        name=nc.get_next_instruction_name(), func=func, ins=inputs,
        outs=[eng.lower_ap(out)]))


def _build_program(tiles, cls_idx, win_offs, Sw):
    import concourse.bacc as bacc
    import concourse.mybir as mybir
    from concourse.tile import TileContext
    from concourse.tile_rust import add_dep_helper
    from concourse.alu_op_type import AluOpType
    from concourse.dve_ops import RECIP_APPROX_FAST_CONSTS as RC
    import concourse.dve_ops as dve_ops

    f32 = mybir.dt.float32
    f16 = mybir.dt.float16
    f8 = mybir.dt.float8e4
    AF = mybir.ActivationFunctionType
    DR = mybir.MatmulPerfMode.DoubleRow
    taylor_den = _register_taylor_den16()
    recip_fast = next(o for o in dve_ops.OPS if o.name == "RECIPROCAL_APPROX_FAST")
    _pin_exp_ln_table()

    nc = bacc.Bacc()
    n_tiles = len(tiles)
    n_win_rows = int(win_offs[-1]) // 2
    xwin = nc.declare_dram_parameter("xwin", [2 * n_win_rows, _SLICE], f16,
                                     isOutput=False)
    wcpack_d = nc.declare_dram_parameter("wcpack", [128, 128 * n_tiles], f8,
                                         isOutput=False)
    w1pack_d = nc.declare_dram_parameter("w1pack", [45, 512], f16, isOutput=False)
    cst32_d = nc.declare_dram_parameter("cst32", [128, 45], f32, isOutput=False)
    fw2t_d = nc.declare_dram_parameter("fw2t", [128, 10], f16, isOutput=False)
    out_d = nc.declare_dram_parameter("out", [_PC, 10], f32, isOutput=True)

    zscale = 1.5 / (16.0 * Sw)

    with TileContext(nc) as tc:
        with (
            tc.tile_pool(name="const", bufs=1) as cpool,
            tc.tile_pool(name="xw", bufs=12) as xpool,
            tc.tile_pool(name="work", bufs=4) as wpool,
            tc.tile_pool(name="cps", bufs=3, space="PSUM") as cps,
            tc.tile_pool(name="zps", bufs=2, space="PSUM") as zps,
        ):
            # w1pack loads via the GpSimd queue so its issue overlaps the
            # Sync queue's first window DMAs; cst32's DMA is emitted inside
            # the loop after the first windows (needed only by the first
            # taylor op at ~11us)
            w1pack_sb = cpool.tile_from(w1pack_d[:], name="w1pack_sb",
                                        forced_dma_engine=mybir.EngineType.Pool)
            cst32_sb = cpool.tile([128, 45], f32, tag="cst32",
                                  name="cst32_sb", bufs=1)
            biaspack_sb = cst32_sb[:, 0:4]
            nbcomb_sb = cst32_sb[:, 4:5]
            fb2r_sb = cst32_sb[:, 5:45]
            # wcpack/fw2t SBUF space is allocated now but the (large) wcpack
            # DMA is deferred until after the first window loads, and fw2t's
            # until the tail, so the PE can start conv1 early.
            wcpack_sb = cpool.tile([128, 128 * n_tiles], f8, tag="wcpack",
                                   name="wcpack_sb", bufs=1)
            fw2t_sb = cpool.tile([128, 10], f16, tag="fw2t", name="fw2t_sb",
                                 bufs=1)

            # single-sync-wait rule: pre-observe PE-read const queues with
            # dummy 1-col matmuls; DVE/ACT-read consts with dummy touches.
            # The dummies write into a slice of the cp psum rotation.
            dps = cps.tile([128, 2 * _SLICE], f32, tag="cp", name="dps")
            nc.tensor.matmul(dps[0:128, 0:1], w1pack_sb[0:45, 0:128],
                             w1pack_sb[0:45, 0:1], start=True, stop=True)
            dvescr = wpool.tile([128, 45], f32, tag="dvescr", name="dvescr", bufs=1)
            nc.vector.tensor_copy(out=dvescr[:], in_=cst32_sb[:])
            actscr = wpool.tile([128, 1], f32, tag="actscr", name="actscr", bufs=1)
            nc.scalar.copy(out=actscr[:], in_=nbcomb_sb)

            zs = []
            # super-groups of 4 tiles (2 psum pairs); a trailing group of 2
            # when n_tiles % 4 == 2
            gbounds = list(range(0, n_tiles, 4))
            groups = [(b, min(4, n_tiles - b)) for b in gbounds]
            recip_insts = []

            def emit_dr(z, s, gbase, ntg):
                # fp8 DoubleRow matmuls for one group, contracting chunk
                # pairs 256-deep into z
                for dr in range(ntg // 2):
                    i0 = gbase + 2 * dr
                    lhsT = wcpack_sb[0:128, 128 * i0:128 * i0 + 256] \
                        .rearrange("p (i m) -> p i m", i=2)
                    rhs = s[:, dr * 2 * _SLICE:(dr + 1) * 2 * _SLICE] \
                        .rearrange("p (i n) -> p i n", i=2)
                    nc.tensor.matmul(z, lhsT, rhs,
                                     start=(i0 == 0),
                                     stop=(i0 + 2 == n_tiles),
                                     perf_mode=DR)

            # software pipelining: group g's DR matmuls are emitted after
            # group g+1's conv matmuls, so the PE never idles waiting for
            # the reciprocal's s-tile (the next convs fill that latency).
            pending_dr = None
            for sl in range(_NSL):
                z = zps.tile([128, _SLICE], f32, tag="z", name=f"z{sl}")
                zs.append(z)
                for sp, (gbase, ntg) in enumerate(groups):
                    tg = tiles[gbase:gbase + ntg]
                    q = wpool.tile([128, 4 * _SLICE], f32, tag="q",
                                   name=f"q{sl}_{sp}")
                    s = wpool.tile([128, 4 * _SLICE], f8, tag="s",
                                   name=f"s{sl}_{sp}")
                    for half in range(ntg // 2):   # one psum pair per half
                        # alternate the DMA-issue engine per psum pair: the
                        # Sync queue alone spends ~0.8us per dma_start and
                        # becomes a bottleneck; GpSimd is otherwise idle.
                        # Per-half (not per-group) alternation also issues the
                        # first group's windows on both queues in parallel,
                        # shortening the pipeline ramp.
                        dma_eng = nc.sync if (half % 2 == 0) else nc.gpsimd
                        cp = cps.tile([128, 2 * _SLICE], f32, tag="cp",
                                      name=f"cp{sl}_{sp}_{half}")
                        for j in range(2):
                            t = tg[2 * half + j]
                            i = gbase + 2 * half + j
                            xw = xpool.tile([t["K"], _SLICE], f16, tag="xw",
                                            name=f"xw{sl}_{i}")
                            ro = int(win_offs[i]) + sl * t["K"]
                            dma_eng.dma_start(out=xw, in_=xwin[ro:ro + t["K"], :])
                            if sl == 0 and sp == 0 and half == 0 and j == 1:
                                # const loads, after the first window DMAs
                                nc.sync.dma_start(out=cst32_sb,
                                                  in_=cst32_d[:])
                                nc.sync.dma_start(out=wcpack_sb,
                                                  in_=wcpack_d[:])
                            if sl == 0 and sp == 2 and half == 0 and j == 0:
                                # tail-only const, loaded once the head clears
                                nc.sync.dma_start(out=fw2t_sb, in_=fw2t_d[:])
                            ci = cls_idx[t["cls"]]
                            nc.tensor.matmul(
                                cp[:, j * _SLICE:(j + 1) * _SLICE],
                                w1pack_sb[0:t["K"], 128 * ci:128 * ci + 128], xw,
                                start=True, stop=True)
                        ca = cls_idx[tg[2 * half]["cls"]]
                        cb = cls_idx[tg[2 * half + 1]["cls"]]
                        qh = q[:, half * 2 * _SLICE:(half + 1) * 2 * _SLICE]
                        if ca == cb:
                            nc.vector._custom_dve(
                                taylor_den, out=qh, in0=cp,
                                s0=biaspack_sb[0:128, ca:ca + 1], s1=2.0, imm2=3.0)
                        else:
                            nc.vector._custom_dve(
                                taylor_den, out=qh[:, 0:_SLICE],
                                in0=cp[:, 0:_SLICE],
                                s0=biaspack_sb[0:128, ca:ca + 1], s1=2.0, imm2=3.0)
                            nc.vector._custom_dve(
                                taylor_den, out=qh[:, _SLICE:2 * _SLICE],
                                in0=cp[:, _SLICE:2 * _SLICE],
                                s0=biaspack_sb[0:128, cb:cb + 1], s1=2.0, imm2=3.0)
                    if sl == 0 and sp == 0:
                        # pre-observe the wcpack PE-read queue only after the
                        # first conv matmuls, so the PE isn't head-blocked on
                        # the wcpack DMA.
                        nc.tensor.matmul(dps[0:128, 1:2],
                                         wcpack_sb[0:128, 0:128],
                                         wcpack_sb[0:128, 0:1],
                                         start=True, stop=True)
                    # s8 = 16/den16 in fp8e4m3 (range (0, ~8])
                    ri = _act_raw(nc, s[:, 0:ntg * _SLICE], q[:, 0:ntg * _SLICE],
                                  AF.Reciprocal, bias=0.0, scale=1.0 / 16.0)
                    recip_insts.append(ri)
                    if pending_dr is not None:
                        emit_dr(*pending_dr)
                    pending_dr = (z, s, gbase, ntg)
            emit_dr(*pending_dr)
            # pre-observe fw2t's PE-read queue before its first real use
            dps2 = cps.tile([128, 2 * _SLICE], f32, tag="cp", name="dps2")
            nc.tensor.matmul(dps2[0:10, 0:1], fw2t_sb[0:128, 0:10],
                             fw2t_sb[0:128, 0:1], start=True, stop=True)
            # ---- tail: sigmoid(fc1) via exp + DVE 1/(1+e), fc2, log_softmax
            # (no max-sub: |logits| < 12, exp cannot overflow fp32). All tail
            # ScalarE work is Exp/Ln (one natural_log_exp table load); order
            # it after the last reciprocal so the table sets load once each.
            last_recip = recip_insts[-1]
            hs = []
            for sl in range(_NSL):
                e1 = wpool.tile([128, _SLICE], f32, tag="e1", name=f"e1{sl}")
                ei = nc.scalar.activation(e1, zs[sl], AF.Exp,
                                          bias=nbcomb_sb[:], scale=-zscale)
                add_dep_helper(ei.ins, last_recip.ins, sync=False,
                               reason="keep tail ACT after recips (table sets)")
                ep = wpool.tile([128, _SLICE], f32, tag="ep", name=f"ep{sl}")
                # +1 on ScalarE's free affine (Copy keeps immediate bias);
                # keeps the DVE free for the reciprocal
                nc.scalar.activation(ep, e1, AF.Copy, bias=1.0, scale=1.0)
                h = wpool.tile([128, _SLICE], f16, tag="h", name=f"h{sl}")
                nc.vector._custom_dve(recip_fast, out=h, in0=ep,
                                      s0=RC["s0"], s1=RC["s1"], imm2=RC["imm2"])
                hs.append(h)
            ng = _SLICE // 128
            ot = wpool.tile([128, _NSL * 10 * ng], f32, tag="ot", name="ot",
                            bufs=1)
            for sl in range(_NSL):
                fpt = zps.tile([128, _SLICE], f32, tag="z", name=f"fpt{sl}")
                fp = fpt[:, 0:10 * ng]
                for g in range(ng):
                    nc.tensor.matmul(fp[:, g * 10:(g + 1) * 10],
                                     hs[sl][:, g * 128:(g + 1) * 128], fw2t_sb[:],
                                     start=True, stop=True)
                lg = wpool.tile([128, 10 * ng], f32, tag="lg", name=f"lg{sl}")
                nc.vector.tensor_tensor(out=lg, in0=fp, in1=fb2r_sb[:, 0:10 * ng],
                                        op=AluOpType.add)
                e = wpool.tile([128, 10 * ng], f32, tag="e", name=f"e{sl}")
                ei = nc.scalar.activation(e, lg, AF.Exp)
                add_dep_helper(ei.ins, last_recip.ins, sync=False,
                               reason="keep tail ACT after recips (table sets)")
                ssum = wpool.tile([128, ng], f32, tag="ss", name=f"ss{sl}")
                nc.vector.tensor_reduce(
                    ssum, e.rearrange("p (g k) -> p g k", k=10),
                    axis=mybir.AxisListType.X, op=AluOpType.add)
                lns = wpool.tile([128, ng], f32, tag="ls", name=f"ls{sl}")
                li = nc.scalar.activation(lns, ssum, AF.Ln)
                add_dep_helper(li.ins, last_recip.ins, sync=False,
                               reason="keep tail ACT after recips (table sets)")
                for g in range(ng):
                    # split the final subtracts across DVE and GpSimd so the
                    # last block runs ~2x wide
                    seng = nc.vector if g % 2 == 0 else nc.gpsimd
                    seng.tensor_scalar(
                        out=ot[:, sl * 10 * ng + g * 10:sl * 10 * ng + (g + 1) * 10],
                        in0=lg[:, g * 10:(g + 1) * 10],
                        scalar1=lns[:, g:g + 1], scalar2=None,
                        op0=AluOpType.subtract)
                # per-slice output DMA on alternating queues: slice 0's
                # transfer overlaps slice 1's tail chain
                oeng = nc.gpsimd if sl == 0 else nc.sync
                orow = sl * _SLICE
                oeng.dma_start(
                    out=out_d[orow:orow + _SLICE, :].rearrange(
                        "(g p) k -> p g k", p=128),
                    in_=ot[:, sl * 10 * ng:(sl + 1) * 10 * ng].rearrange(
                        "p (g k) -> p g k", k=10))
    nc.compile()
    return nc


_PROGRAM_CACHE = {}


def kernel(x, w1, b1, w2, b2, fw1, fb1, fw2, fb2):
    global LAST_RESULTS
    wins, consts, tiles = _host_prep(x, w1, b1, w2, b2, fw1, fb1, fw2, fb2)

    if "nc" not in _PROGRAM_CACHE:
        _PROGRAM_CACHE["nc"] = _build_program(tiles, consts["cls_idx"],
                                              consts["win_offs"], consts["Sw"])
    nc = _PROGRAM_CACHE["nc"]

    shared = {k: consts[k] for k in
              ("wcpack", "w1pack", "cst32", "fw2t")}
    in_maps = []
    for c in range(_NCORES):
        m = dict(shared)
        # per-core pre-windowed blob: per tile, per slice, [K_t, 512] blocks
        blocks = []
        for t_i, t in enumerate(tiles):
            w = wins[t_i][:, c * _PC:(c + 1) * _PC]
            for sl in range(_NSL):
                blocks.append(w[:, sl * _SLICE:(sl + 1) * _SLICE])
        m["xwin"] = np.ascontiguousarray(np.concatenate(blocks, axis=0))
        in_maps.append(m)

    from concourse.bass_utils import run_bass_kernel_spmd
    trace = bool(int(os.environ.get("BASS_KERNEL_TRACE", "0")))
    res = run_bass_kernel_spmd(nc, in_maps, core_ids=list(range(_NCORES)),
                               trace=trace)
    LAST_RESULTS = res
    return np.concatenate([r["out"] for r in res.results], axis=0)


# revision 27
# speedup vs baseline: 1.0119x; 1.0119x over previous
"""Trainium2 Bass kernel for nn_Net_39230231281866 (dense_cnn).

Network: conv3x3(1->6) -> Taylor-sigmoid -> conv3x3(6->7) -> flatten
         -> fc(4032->128) -> sigmoid -> fc(128->10) -> log_softmax,
batch 8192, data-parallel over 8 NeuronCores (1024 samples/core).

Mapping:
  * conv2+fc1 folded on the host into one dense GEMM W_comb [128, 4056].
  * conv1 = banded-weight matmul (K = input-pixel window, M <= 128 output
    positions x channels, batch on the moving free dim), fp16, tiled as
    34 dense chunks (24x(3,7) + 6x(4,5) + 3x(2,8) + 1x(2,2)).
  * The folded GEMM runs in fp8e4m3 with MatmulPerfMode.DoubleRow: chunk
    pairs are contracted 256-deep per instruction, halving the second
    GEMM's matmul count. W_comb is scaled by a power of two (Sw) into
    fp8 range; the inverse scale rides the fc1-sigmoid exp's free affine.
  * Taylor-sigmoid: custom DVE op computes den16(u) = u^4+2u^3+3u^2+3u+3
    where u = (-conv1(x)-b1)/2 (scale folded into the conv weights); the
    reciprocal runs on ScalarE (Reciprocal PWP table) as s8 = 16/den16,
    written directly as fp8e4m3 (range ~(0, 8] keeps fp8 normals).
  * fc1 sigmoid = exp on ScalarE (natural_log_exp table set, shared with
    the log_softmax tail -> one table switch total) + 1/(1+e) on the DVE
    via RECIPROCAL_APPROX_FAST.
  * conv1 outputs land in PSUM-bank pairs (triple-buffered, 6 of 8 banks)
    so elementwise ops run at free-dim 1024; the z accumulator and the
    fc2 psum share the remaining 2 banks.
  * The PE stream is software-pipelined: group g's DoubleRow matmuls are
    emitted after group g+1's conv matmuls, hiding the taylor+reciprocal
    latency so the PE never stalls on the s-tile.
  * dma_start issue costs ~0.8us on an engine queue, so window-DMA issue
    alternates between the Sync and (otherwise idle) GpSimd queues; the
    big fp8 wcpack load is deferred until after the first window DMAs and
    its pre-observe dummy matmul is emitted after the first conv group.
"""

import os
import numpy as np
import ml_dtypes

_B = 8192
_NCORES = 8
_PC = _B // _NCORES
_SLICE = 512
_NSL = _PC // _SLICE

LAST_RESULTS = None


def _tiles():
    """conv1 output tiling, 34 tiles (vs naive 36): 24x(3,7) M=126 +
    6x(4,5) M=120 + 3x(2,8) M=96 + 1x(2,2) M=24 -- denser M packing cuts
    PE/DVE/ACT work ~6%. Sorted by class so psum-pair mates share a bias
    vector; the final pair is mixed-class (handled as two half DVE ops)."""
    specs = []
    for oy0 in range(0, 24, 3):
        for ox0 in (0, 7, 14):
            specs.append((oy0, 3, ox0, 7))
    for oy0 in range(0, 24, 4):
        specs.append((oy0, 4, 21, 5))
    for ox0 in (0, 8, 16):
        specs.append((24, 2, ox0, 8))
    specs.append((24, 2, 24, 2))
    ts = []
    for (oy0, noy, ox0, nox) in specs:
        ts.append(dict(oy0=oy0, noy=noy, ox0=ox0, nox=nox,
                       ky=noy + 2, kx=nox + 2,
                       K=(noy + 2) * (nox + 2), M=noy * nox * 6,
                       cls=(noy, nox)))
    order = {(3, 7): 0, (4, 5): 1, (2, 8): 2, (2, 2): 3}
    ts.sort(key=lambda t: order[t["cls"]])
    return ts


def _host_prep(x, w1, b1, w2, b2, fw1, fb1, fw2, fb2):
    x = np.asarray(x, np.float32)
    w1 = np.asarray(w1, np.float32); b1 = np.asarray(b1, np.float32)
    w2 = np.asarray(w2, np.float32); b2 = np.asarray(b2, np.float32)
    fw1 = np.asarray(fw1, np.float32); fb1 = np.asarray(fb1, np.float32)
    fw2 = np.asarray(fw2, np.float32); fb2 = np.asarray(fb2, np.float32)

    tiles = _tiles()

    # banded conv1 weights, scaled by -1/2 (u = (-conv-b1)/2), M padded to 128
    cls_list = [(3, 7), (4, 5), (2, 8), (2, 2)]
    cls_idx = {c: i for i, c in enumerate(cls_list)}
    w1pack = np.zeros((45, 128 * 4), np.float32)
    biaspack = np.zeros((128, 4), np.float32)
    for cls in cls_list:
        noy, nox = cls
        kx = nox + 2
        ci = cls_idx[cls]
        for oy in range(noy):
            for ox in range(nox):
                for oc in range(6):
                    m = (oy * nox + ox) * 6 + oc
                    biaspack[m, ci] = -0.5 * b1[oc]
                    for dy in range(3):
                        for dx in range(3):
                            k = (oy + dy) * kx + (ox + dx)
                            w1pack[k, 128 * ci + m] = -0.5 * w1[oc, 0, dy, dx]

    # fold conv2 + fc1 -> W_comb [128, 6*26*26], b_comb. The reference's
    # 1.5 factor (s = 1.5/den16) moves into the z unscale: the kernel
    # computes s8 = 16/den16, so z_true = z_psum * 1.5/(16*Sw).
    fw1r = fw1.reshape(128, 7, 24, 24)
    Wc = np.zeros((128, 6, 26, 26), np.float32)
    for dy in range(3):
        for dx in range(3):
            Wc[:, :, dy:dy + 24, dx:dx + 24] += np.einsum(
                "joyx,oi->jiyx", fw1r, w2[:, :, dy, dx], optimize=True)
    b_comb = fb1 + np.einsum("joyx,o->j", fw1r, b2)
    Wc_flat = Wc.reshape(128, 6 * 26 * 26)

    # fp8 scale: power of two that puts max|Wc| around 240 (e4m3 max 448)
    Sw = float(2.0 ** np.floor(np.log2(240.0 / np.abs(Wc_flat).max())))

    # W_comb columns in conv1-chunk partition order, packed [128, 36*128] fp8
    f8 = ml_dtypes.float8_e4m3fn
    wcpack = np.zeros((128, 128 * len(tiles)), np.float32)
    for t_i, t in enumerate(tiles):
        rows = []
        for oy in range(t["noy"]):
            for ox in range(t["nox"]):
                for oc in range(6):
                    rows.append((oc * 26 + t["oy0"] + oy) * 26 + t["ox0"] + ox)
        wcpack[:t["M"], 128 * t_i:128 * t_i + 128] = (Sw * Wc_flat)[:, rows].T

    f16 = np.float16
    # one f32 const blob: [0:4] taylor bias per class, [4:5] -b_comb,
    # [5:45] fb2 tiled x4 -- single DMA instead of three
    cst32 = np.zeros((128, 45), np.float32)
    cst32[:, 0:4] = biaspack
    cst32[:, 4] = -b_comb
    cst32[:, 5:45] = np.tile(fb2.reshape(1, 10), (128, 4))
    consts = dict(
        wcpack=wcpack.astype(f8), w1pack=w1pack.astype(f16),
        cst32=cst32, cls_idx=cls_idx, Sw=Sw,
        fw2t=np.ascontiguousarray(fw2.T).astype(f16),                   # [128, 10]
    )
    # pre-windowed input: for each conv tile a contiguous [K_t, B] block of
    # pixel-major rows, so each per-slice window DMA is one dense 2D transfer
    x_pm = x.reshape(_B, 784).T.astype(f16)                             # [784, B]
    wins = []
    for t in tiles:
        rows = (np.arange(t["ky"])[:, None] + t["oy0"]) * 28 + \
               (np.arange(t["kx"])[None, :] + t["ox0"])
        wins.append(x_pm[rows.reshape(-1), :])                          # [K_t, B]
    consts["win_offs"] = np.cumsum([0] + [2 * t["K"] for t in tiles])
    return wins, consts, tiles


def _register_taylor_den16():
    import concourse.dve_ops as dve_ops
    if "TAYLOR_DEN16_ANT" in dve_ops._SUB_OPCODE_FOR_NAME:
        return next(o for o in dve_ops.OPS if o.name == "TAYLOR_DEN16_ANT")
    from concourse.dve_spec import Spec, Src0, C0, C1, C2

    # u = in0 + s0;  out = u^4 + 2u^3 + 3u^2 + 3u + 3  ==  (q(t)+48)/16
    u = Src0 + C0
    body = ((((u + C1) * u + C2) * u + C2) * u + C2)

    def _ref(in0, in1, s0, s1, imm2):
        xx = in0.astype(np.float32) + s0
        return (((xx + s1) * xx + imm2) * xx + imm2) * xx + imm2

    op = dve_ops.DveOp(
        "TAYLOR_DEN16_ANT",
        Spec(body=body, reference=_ref),
        subdim=False,
        uops_sha={"v3": "0d84493259836d20", "v4": "be052b2c26b42830"},
    )
    dve_ops.OPS.append(op)
    dve_ops.CUSTOM_DVE_SPECS[op.name] = op.spec
    row = max(dve_ops._SUB_OPCODE_FOR_NAME.values()) + 1
    assert row < 0x20
    dve_ops._SUB_OPCODE_FOR_NAME[op.name] = row
    return op


def _pin_exp_ln_table():
    """Make Exp and Ln resolve only to natural_log_exp_and_others so the
    whole post-reciprocal tail costs one table load."""
    import concourse.bacc as bacc
    import concourse.mybir as mybir
    if getattr(bacc, "_ant_expln_pinned", False):
        return
    orig = bacc.get_activation_tables
    AF = mybir.ActivationFunctionType

    def patched(arch):
        tabs = {k: set(v) for k, v in orig(arch).items()}
        for name, fns in tabs.items():
            if name != "natural_log_exp_and_others":
                fns.discard(AF.Exp)
                fns.discard(AF.Ln)
        return tabs

    bacc.get_activation_tables = patched
    bacc._ant_expln_pinned = True


def _act_raw(nc, out, in_, func, bias=0.0, scale=1.0):
    """Emit InstActivation directly (used for Reciprocal, which the
    nc.scalar.activation wrapper refuses; measured ~1.2e-5 rel err)."""
    import concourse.mybir as mybir
    eng = nc.scalar
    inputs = [eng.lower_ap(in_)]
    for arg in (bias, scale, 0.0):
        inputs.append(mybir.ImmediateValue(dtype=mybir.dt.float32,
                                           value=float(arg)))
    return eng.add_instruction(mybir.InstActivation(
        name=nc.get_next_instruction_name(), func=func, ins=inputs,
        outs=[eng.lower_ap(out)]))


def _build_program(tiles, cls_idx, win_offs, Sw):
    import concourse.bacc as bacc
    import concourse.mybir as mybir
    from concourse.tile import TileContext
    from concourse.tile_rust import add_dep_helper
    from concourse.alu_op_type import AluOpType
    from concourse.dve_ops import RECIP_APPROX_FAST_CONSTS as RC
    import concourse.dve_ops as dve_ops

    f32 = mybir.dt.float32
    f16 = mybir.dt.float16
    f8 = mybir.dt.float8e4
    AF = mybir.ActivationFunctionType
    DR = mybir.MatmulPerfMode.DoubleRow
    taylor_den = _register_taylor_den16()
    recip_fast = next(o for o in dve_ops.OPS if o.name == "RECIPROCAL_APPROX_FAST")
    _pin_exp_ln_table()

    nc = bacc.Bacc()
    n_tiles = len(tiles)
    n_win_rows = int(win_offs[-1]) // 2
    xwin = nc.declare_dram_parameter("xwin", [2 * n_win_rows, _SLICE], f16,
                                     isOutput=False)
    wcpack_d = nc.declare_dram_parameter("wcpack", [128, 128 * n_tiles], f8,
                                         isOutput=False)
    w1pack_d = nc.declare_dram_parameter("w1pack", [45, 512], f16, isOutput=False)
    cst32_d = nc.declare_dram_parameter("cst32", [128, 45], f32, isOutput=False)
    fw2t_d = nc.declare_dram_parameter("fw2t", [128, 10], f16, isOutput=False)
    out_d = nc.declare_dram_parameter("out", [_PC, 10], f32, isOutput=True)

    zscale = 1.5 / (16.0 * Sw)

    with TileContext(nc) as tc:
        with (
            tc.tile_pool(name="const", bufs=1) as cpool,
            tc.tile_pool(name="xw", bufs=12) as xpool,
            tc.tile_pool(name="work", bufs=4) as wpool,
            tc.tile_pool(name="cps", bufs=3, space="PSUM") as cps,
            tc.tile_pool(name="zps", bufs=2, space="PSUM") as zps,
        ):
            # head consts load via the GpSimd queue so their issue overlaps
            # the Sync queue's first window DMAs
            w1pack_sb = cpool.tile_from(w1pack_d[:], name="w1pack_sb",
                                        forced_dma_engine=mybir.EngineType.Pool)
            cst32_sb = cpool.tile_from(cst32_d[:], name="cst32_sb",
                                       forced_dma_engine=mybir.EngineType.Pool)
            biaspack_sb = cst32_sb[:, 0:4]
            nbcomb_sb = cst32_sb[:, 4:5]
            fb2r_sb = cst32_sb[:, 5:45]
            # wcpack/fw2t SBUF space is allocated now but the (large) wcpack
            # DMA is deferred until after the first window loads, and fw2t's
            # until the tail, so the PE can start conv1 early.
            wcpack_sb = cpool.tile([128, 128 * n_tiles], f8, tag="wcpack",
                                   name="wcpack_sb", bufs=1)
            fw2t_sb = cpool.tile([128, 10], f16, tag="fw2t", name="fw2t_sb",
                                 bufs=1)

            # single-sync-wait rule: pre-observe PE-read const queues with
            # dummy 1-col matmuls; DVE/ACT-read consts with dummy touches.
            # The dummies write into a slice of the cp psum rotation.
            dps = cps.tile([128, 2 * _SLICE], f32, tag="cp", name="dps")
            nc.tensor.matmul(dps[0:128, 0:1], w1pack_sb[0:45, 0:128],
                             w1pack_sb[0:45, 0:1], start=True, stop=True)
            dvescr = wpool.tile([128, 45], f32, tag="dvescr", name="dvescr", bufs=1)
            nc.vector.tensor_copy(out=dvescr[:], in_=cst32_sb[:])
            actscr = wpool.tile([128, 1], f32, tag="actscr", name="actscr", bufs=1)
            nc.scalar.copy(out=actscr[:], in_=nbcomb_sb)

            zs = []
            # super-groups of 4 tiles (2 psum pairs); a trailing group of 2
            # when n_tiles % 4 == 2
            gbounds = list(range(0, n_tiles, 4))
            groups = [(b, min(4, n_tiles - b)) for b in gbounds]
            recip_insts = []

            def emit_dr(z, s, gbase, ntg):
                # fp8 DoubleRow matmuls for one group, contracting chunk
                # pairs 256-deep into z
                for dr in range(ntg // 2):
                    i0 = gbase + 2 * dr
                    lhsT = wcpack_sb[0:128, 128 * i0:128 * i0 + 256] \
                        .rearrange("p (i m) -> p i m", i=2)
                    rhs = s[:, dr * 2 * _SLICE:(dr + 1) * 2 * _SLICE] \
                        .rearrange("p (i n) -> p i n", i=2)
                    nc.tensor.matmul(z, lhsT, rhs,
                                     start=(i0 == 0),
                                     stop=(i0 + 2 == n_tiles),
                                     perf_mode=DR)

            # software pipelining: group g's DR matmuls are emitted after
            # group g+1's conv matmuls, so the PE never idles waiting for
            # the reciprocal's s-tile (the next convs fill that latency).
            pending_dr = None
            for sl in range(_NSL):
                z = zps.tile([128, _SLICE], f32, tag="z", name=f"z{sl}")
                zs.append(z)
                for sp, (gbase, ntg) in enumerate(groups):
                    tg = tiles[gbase:gbase + ntg]
                    q = wpool.tile([128, 4 * _SLICE], f32, tag="q",
                                   name=f"q{sl}_{sp}")
                    s = wpool.tile([128, 4 * _SLICE], f8, tag="s",
                                   name=f"s{sl}_{sp}")
                    for half in range(ntg // 2):   # one psum pair per half
                        # alternate the DMA-issue engine per psum pair: the
                        # Sync queue alone spends ~0.8us per dma_start and
                        # becomes a bottleneck; GpSimd is otherwise idle.
                        # Per-half (not per-group) alternation also issues the
                        # first group's windows on both queues in parallel,
                        # shortening the pipeline ramp.
                        dma_eng = nc.sync if (half % 2 == 0) else nc.gpsimd
                        cp = cps.tile([128, 2 * _SLICE], f32, tag="cp",
                                      name=f"cp{sl}_{sp}_{half}")
                        for j in range(2):
                            t = tg[2 * half + j]
                            i = gbase + 2 * half + j
                            xw = xpool.tile([t["K"], _SLICE], f16, tag="xw",
                                            name=f"xw{sl}_{i}")
                            ro = int(win_offs[i]) + sl * t["K"]
                            dma_eng.dma_start(out=xw, in_=xwin[ro:ro + t["K"], :])
                            if sl == 0 and sp == 0 and half == 0 and j == 1:
                                # big const load, after the first window DMAs
                                nc.sync.dma_start(out=wcpack_sb,
                                                  in_=wcpack_d[:])
                            if sl == 0 and sp == 2 and half == 0 and j == 0:
                                # tail-only const, loaded once the head clears
                                nc.sync.dma_start(out=fw2t_sb, in_=fw2t_d[:])
                            ci = cls_idx[t["cls"]]
                            nc.tensor.matmul(
                                cp[:, j * _SLICE:(j + 1) * _SLICE],
                                w1pack_sb[0:t["K"], 128 * ci:128 * ci + 128], xw,
                                start=True, stop=True)
                        ca = cls_idx[tg[2 * half]["cls"]]
                        cb = cls_idx[tg[2 * half + 1]["cls"]]
                        qh = q[:, half * 2 * _SLICE:(half + 1) * 2 * _SLICE]
                        if ca == cb:
                            nc.vector._custom_dve(
                                taylor_den, out=qh, in0=cp,
                                s0=biaspack_sb[0:128, ca:ca + 1], s1=2.0, imm2=3.0)
                        else:
                            nc.vector._custom_dve(
                                taylor_den, out=qh[:, 0:_SLICE],
                                in0=cp[:, 0:_SLICE],
                                s0=biaspack_sb[0:128, ca:ca + 1], s1=2.0, imm2=3.0)
                            nc.vector._custom_dve(
                                taylor_den, out=qh[:, _SLICE:2 * _SLICE],
                                in0=cp[:, _SLICE:2 * _SLICE],
                                s0=biaspack_sb[0:128, cb:cb + 1], s1=2.0, imm2=3.0)
                    if sl == 0 and sp == 0:
                        # pre-observe the wcpack PE-read queue only after the
                        # first conv matmuls, so the PE isn't head-blocked on
                        # the wcpack DMA.
                        nc.tensor.matmul(dps[0:128, 1:2],
                                         wcpack_sb[0:128, 0:128],
                                         wcpack_sb[0:128, 0:1],
                                         start=True, stop=True)
                    # s8 = 16/den16 in fp8e4m3 (range (0, ~8])
                    ri = _act_raw(nc, s[:, 0:ntg * _SLICE], q[:, 0:ntg * _SLICE],
                                  AF.Reciprocal, bias=0.0, scale=1.0 / 16.0)
                    recip_insts.append(ri)
                    if pending_dr is not None:
                        emit_dr(*pending_dr)
                    pending_dr = (z, s, gbase, ntg)
            emit_dr(*pending_dr)
            # pre-observe fw2t's PE-read queue before its first real use
            dps2 = cps.tile([128, 2 * _SLICE], f32, tag="cp", name="dps2")
            nc.tensor.matmul(dps2[0:10, 0:1], fw2t_sb[0:128, 0:10],
                             fw2t_sb[0:128, 0:1], start=True, stop=True)
            # ---- tail: sigmoid(fc1) via exp + DVE 1/(1+e), fc2, log_softmax
            # (no max-sub: |logits| < 12, exp cannot overflow fp32). All tail
            # ScalarE work is Exp/Ln (one natural_log_exp table load); order
            # it after the last reciprocal so the table sets load once each.
            last_recip = recip_insts[-1]
            hs = []
            for sl in range(_NSL):
                e1 = wpool.tile([128, _SLICE], f32, tag="e1", name=f"e1{sl}")
                ei = nc.scalar.activation(e1, zs[sl], AF.Exp,
                                          bias=nbcomb_sb[:], scale=-zscale)
                add_dep_helper(ei.ins, last_recip.ins, sync=False,
                               reason="keep tail ACT after recips (table sets)")
                ep = wpool.tile([128, _SLICE], f32, tag="ep", name=f"ep{sl}")
                # +1 on the DVE: the tail's critical engine is ScalarE (the
                # two exps), so the add must stay off its queue
                nc.vector.tensor_scalar(out=ep, in0=e1, scalar1=1.0,
                                        scalar2=None, op0=AluOpType.add)
                h = wpool.tile([128, _SLICE], f16, tag="h", name=f"h{sl}")
                nc.vector._custom_dve(recip_fast, out=h, in0=ep,
                                      s0=RC["s0"], s1=RC["s1"], imm2=RC["imm2"])
                hs.append(h)
            ng = _SLICE // 128
            ot = wpool.tile([128, _NSL * 10 * ng], f32, tag="ot", name="ot",
                            bufs=1)
            for sl in range(_NSL):
                fpt = zps.tile([128, _SLICE], f32, tag="z", name=f"fpt{sl}")
                fp = fpt[:, 0:10 * ng]
                for g in range(ng):
                    nc.tensor.matmul(fp[:, g * 10:(g + 1) * 10],
                                     hs[sl][:, g * 128:(g + 1) * 128], fw2t_sb[:],
                                     start=True, stop=True)
                lg = wpool.tile([128, 10 * ng], f32, tag="lg", name=f"lg{sl}")
                nc.vector.tensor_tensor(out=lg, in0=fp, in1=fb2r_sb[:, 0:10 * ng],
                                        op=AluOpType.add)
                e = wpool.tile([128, 10 * ng], f32, tag="e", name=f"e{sl}")
                ei = nc.scalar.activation(e, lg, AF.Exp)
                add_dep_helper(ei.ins, last_recip.ins, sync=False,
                               reason="keep tail ACT after recips (table sets)")
                ssum = wpool.tile([128, ng], f32, tag="ss", name=f"ss{sl}")
                nc.vector.tensor_reduce(
                    ssum, e.rearrange("p (g k) -> p g k", k=10),
                    axis=mybir.AxisListType.X, op=AluOpType.add)
                lns = wpool.tile([128, ng], f32, tag="ls", name=f"ls{sl}")
                li = nc.scalar.activation(lns, ssum, AF.Ln)
                add_dep_helper(li.ins, last_recip.ins, sync=False,
                               reason="keep tail ACT after recips (table sets)")
                for g in range(ng):
                    nc.vector.tensor_scalar(
                        out=ot[:, sl * 10 * ng + g * 10:sl * 10 * ng + (g + 1) * 10],
                        in0=lg[:, g * 10:(g + 1) * 10],
                        scalar1=lns[:, g:g + 1], scalar2=None,
                        op0=AluOpType.subtract)
            # one output DMA for the whole core: out rows = sl*512 + g*128 + p
            nc.sync.dma_start(
                out=out_d[:].rearrange("(s g p) k -> p s g k", p=128, s=_NSL),
                in_=ot.rearrange("p (s g k) -> p s g k", k=10, s=_NSL))
    nc.compile()
    return nc


_PROGRAM_CACHE = {}


def kernel(x, w1, b1, w2, b2, fw1, fb1, fw2, fb2):
    global LAST_RESULTS
    wins, consts, tiles = _host_prep(x, w1, b1, w2, b2, fw1, fb1, fw2, fb2)

    if "nc" not in _PROGRAM_CACHE:
        _PROGRAM_CACHE["nc"] = _build_program(tiles, consts["cls_idx"],
                                              consts["win_offs"], consts["Sw"])
    nc = _PROGRAM_CACHE["nc"]

    shared = {k: consts[k] for k in
              ("wcpack", "w1pack", "cst32", "fw2t")}
    in_maps = []
    for c in range(_NCORES):
        m = dict(shared)
        # per-core pre-windowed blob: per tile, per slice, [K_t, 512] blocks
        blocks = []
        for t_i, t in enumerate(tiles):
            w = wins[t_i][:, c * _PC:(c + 1) * _PC]
            for sl in range(_NSL):
                blocks.append(w[:, sl * _SLICE:(sl + 1) * _SLICE])
        m["xwin"] = np.ascontiguousarray(np.concatenate(blocks, axis=0))
        in_maps.append(m)

    from concourse.bass_utils import run_bass_kernel_spmd
    trace = bool(int(os.environ.get("BASS_KERNEL_TRACE", "0")))
    res = run_bass_kernel_spmd(nc, in_maps, core_ids=list(range(_NCORES)),
                               trace=trace)
    LAST_RESULTS = res
    return np.concatenate([r["out"] for r in res.results], axis=0)


# revision 29
# speedup vs baseline: 1.0538x; 1.0414x over previous
"""Trainium2 Bass kernel for nn_Net_39230231281866 (dense_cnn).

Network: conv3x3(1->6) -> Taylor-sigmoid -> conv3x3(6->7) -> flatten
         -> fc(4032->128) -> sigmoid -> fc(128->10) -> log_softmax,
batch 8192, data-parallel over 8 NeuronCores (1024 samples/core).

Mapping:
  * conv2+fc1 folded on the host into one dense GEMM W_comb [128, 4056].
  * conv1 = banded-weight matmul (K = input-pixel window, M <= 128 output
    positions x channels, batch on the moving free dim), fp16, tiled as
    34 dense chunks (24x(3,7) + 6x(4,5) + 3x(2,8) + 1x(2,2)).
  * The folded GEMM runs in fp8e4m3 with MatmulPerfMode.DoubleRow: chunk
    pairs are contracted 256-deep per instruction, halving the second
    GEMM's matmul count. W_comb is scaled by a power of two (Sw) into
    fp8 range; the inverse scale rides the fc1-sigmoid exp's free affine.
  * Taylor-sigmoid: custom DVE op computes den16(u) = u^4+2u^3+3u^2+3u+3
    where u = (-conv1(x)-b1)/2 (scale folded into the conv weights); the
    reciprocal runs on ScalarE (Reciprocal PWP table) as s8 = 16/den16,
    written directly as fp8e4m3 (range ~(0, 8] keeps fp8 normals).
  * fc1 sigmoid = exp on ScalarE (natural_log_exp table set, shared with
    the log_softmax tail -> one table switch total) + 1/(1+e) on the DVE
    via RECIPROCAL_APPROX_FAST.
  * conv1 outputs land in PSUM-bank pairs (triple-buffered, 6 of 8 banks)
    so elementwise ops run at free-dim 1024; the z accumulator and the
    fc2 psum share the remaining 2 banks.
  * The PE stream is software-pipelined: group g's DoubleRow matmuls are
    emitted after group g+1's conv matmuls, hiding the taylor+reciprocal
    latency so the PE never stalls on the s-tile.
  * dma_start issue costs ~0.8us on an engine queue, so window-DMA issue
    alternates between the Sync and (otherwise idle) GpSimd queues; the
    big fp8 wcpack load is deferred until after the first window DMAs and
    its pre-observe dummy matmul is emitted after the first conv group.
"""

import os
import numpy as np
import ml_dtypes

_B = 8192
_NCORES = 8
_PC = _B // _NCORES
_SLICE = 512
_NSL = _PC // _SLICE

LAST_RESULTS = None


def _tiles():
    """conv1 output tiling, 34 tiles (vs naive 36): 24x(3,7) M=126 +
    6x(4,5) M=120 + 3x(2,8) M=96 + 1x(2,2) M=24 -- denser M packing cuts
    PE/DVE/ACT work ~6%. Sorted by class so psum-pair mates share a bias
    vector; the final pair is mixed-class (handled as two half DVE ops)."""
    specs = []
    for oy0 in range(0, 24, 3):
        for ox0 in (0, 7, 14):
            specs.append((oy0, 3, ox0, 7))
    for oy0 in range(0, 24, 4):
        specs.append((oy0, 4, 21, 5))
    for ox0 in (0, 8, 16):
        specs.append((24, 2, ox0, 8))
    specs.append((24, 2, 24, 2))
    ts = []
    for (oy0, noy, ox0, nox) in specs:
        ts.append(dict(oy0=oy0, noy=noy, ox0=ox0, nox=nox,
                       ky=noy + 2, kx=nox + 2,
                       K=(noy + 2) * (nox + 2), M=noy * nox * 6,
                       cls=(noy, nox)))
    order = {(3, 7): 0, (4, 5): 1, (2, 8): 2, (2, 2): 3}
    ts.sort(key=lambda t: order[t["cls"]])
    return ts


def _host_prep(x, w1, b1, w2, b2, fw1, fb1, fw2, fb2):
    x = np.asarray(x, np.float32)
    w1 = np.asarray(w1, np.float32); b1 = np.asarray(b1, np.float32)
    w2 = np.asarray(w2, np.float32); b2 = np.asarray(b2, np.float32)
    fw1 = np.asarray(fw1, np.float32); fb1 = np.asarray(fb1, np.float32)
    fw2 = np.asarray(fw2, np.float32); fb2 = np.asarray(fb2, np.float32)

    tiles = _tiles()

    # banded conv1 weights, scaled by -1/2 (u = (-conv-b1)/2), M padded to 128
    cls_list = [(3, 7), (4, 5), (2, 8), (2, 2)]
    cls_idx = {c: i for i, c in enumerate(cls_list)}
    w1pack = np.zeros((45, 128 * 4), np.float32)
    biaspack = np.zeros((128, 4), np.float32)
    for cls in cls_list:
        noy, nox = cls
        kx = nox + 2
        ci = cls_idx[cls]
        for oy in range(noy):
            for ox in range(nox):
                for oc in range(6):
                    m = (oy * nox + ox) * 6 + oc
                    biaspack[m, ci] = -0.5 * b1[oc]
                    for dy in range(3):
                        for dx in range(3):
                            k = (oy + dy) * kx + (ox + dx)
                            w1pack[k, 128 * ci + m] = -0.5 * w1[oc, 0, dy, dx]

    # fold conv2 + fc1 -> W_comb [128, 6*26*26], b_comb. The reference's
    # 1.5 factor (s = 1.5/den16) moves into the z unscale: the kernel
    # computes s8 = 16/den16, so z_true = z_psum * 1.5/(16*Sw).
    fw1r = fw1.reshape(128, 7, 24, 24)
    Wc = np.zeros((128, 6, 26, 26), np.float32)
    for dy in range(3):
        for dx in range(3):
            Wc[:, :, dy:dy + 24, dx:dx + 24] += np.einsum(
                "joyx,oi->jiyx", fw1r, w2[:, :, dy, dx], optimize=True)
    b_comb = fb1 + np.einsum("joyx,o->j", fw1r, b2)
    Wc_flat = Wc.reshape(128, 6 * 26 * 26)

    # fp8 scale: power of two that puts max|16*Sw*Wc| around 240 (e4m3 max
    # 448). The LAST group's chunks carry a 16x-larger weight scale because
    # their reciprocal runs on the DVE (RECIPROCAL_APPROX_FAST has no input
    # scale, so it produces 1/den16 instead of ScalarE's 16/den16).
    Sw = float(2.0 ** np.floor(np.log2(15.0 / np.abs(Wc_flat).max())))

    # W_comb columns in conv1-chunk partition order, packed [128, 34*128] fp8
    f8 = ml_dtypes.float8_e4m3fn
    n_last = len(tiles) - (len(tiles) - 1) % 4 - 1  # first tile of last group
    wcpack = np.zeros((128, 128 * len(tiles)), np.float32)
    for t_i, t in enumerate(tiles):
        rows = []
        for oy in range(t["noy"]):
            for ox in range(t["nox"]):
                for oc in range(6):
                    rows.append((oc * 26 + t["oy0"] + oy) * 26 + t["ox0"] + ox)
        sc = 16.0 * Sw if t_i >= n_last else Sw
        wcpack[:t["M"], 128 * t_i:128 * t_i + 128] = (sc * Wc_flat)[:, rows].T

    f16 = np.float16
    # one f32 const blob: [0:4] taylor bias per class, [4:5] -b_comb,
    # [5:45] fb2 tiled x4 -- single DMA instead of three
    cst32 = np.zeros((128, 45), np.float32)
    cst32[:, 0:4] = biaspack
    cst32[:, 4] = -b_comb
    cst32[:, 5:45] = np.tile(fb2.reshape(1, 10), (128, 4))
    consts = dict(
        wcpack=wcpack.astype(f8), w1pack=w1pack.astype(f16),
        cst32=cst32, cls_idx=cls_idx, Sw=Sw,
        fw2t=np.ascontiguousarray(fw2.T).astype(f16),                   # [128, 10]
    )
    # pre-windowed input: for each conv tile a contiguous [K_t, B] block of
    # pixel-major rows, so each per-slice window DMA is one dense 2D transfer
    x_pm = x.reshape(_B, 784).T.astype(f16)                             # [784, B]
    wins = []
    for t in tiles:
        rows = (np.arange(t["ky"])[:, None] + t["oy0"]) * 28 + \
               (np.arange(t["kx"])[None, :] + t["ox0"])
        wins.append(x_pm[rows.reshape(-1), :])                          # [K_t, B]
    consts["win_offs"] = np.cumsum([0] + [2 * t["K"] for t in tiles])
    return wins, consts, tiles


def _register_taylor_den16():
    import concourse.dve_ops as dve_ops
    if "TAYLOR_DEN16_ANT" in dve_ops._SUB_OPCODE_FOR_NAME:
        return next(o for o in dve_ops.OPS if o.name == "TAYLOR_DEN16_ANT")
    from concourse.dve_spec import Spec, Src0, C0, C1, C2

    # u = in0 + s0;  out = u^4 + 2u^3 + 3u^2 + 3u + 3  ==  (q(t)+48)/16
    u = Src0 + C0
    body = ((((u + C1) * u + C2) * u + C2) * u + C2)

    def _ref(in0, in1, s0, s1, imm2):
        xx = in0.astype(np.float32) + s0
        return (((xx + s1) * xx + imm2) * xx + imm2) * xx + imm2

    op = dve_ops.DveOp(
        "TAYLOR_DEN16_ANT",
        Spec(body=body, reference=_ref),
        subdim=False,
        uops_sha={"v3": "0d84493259836d20", "v4": "be052b2c26b42830"},
    )
    dve_ops.OPS.append(op)
    dve_ops.CUSTOM_DVE_SPECS[op.name] = op.spec
    row = max(dve_ops._SUB_OPCODE_FOR_NAME.values()) + 1
    assert row < 0x20
    dve_ops._SUB_OPCODE_FOR_NAME[op.name] = row
    return op


def _pin_exp_ln_table():
    """Make Exp and Ln resolve only to natural_log_exp_and_others so the
    whole post-reciprocal tail costs one table load."""
    import concourse.bacc as bacc
    import concourse.mybir as mybir
    if getattr(bacc, "_ant_expln_pinned", False):
        return
    orig = bacc.get_activation_tables
    AF = mybir.ActivationFunctionType

    def patched(arch):
        tabs = {k: set(v) for k, v in orig(arch).items()}
        for name, fns in tabs.items():
            if name != "natural_log_exp_and_others":
                fns.discard(AF.Exp)
                fns.discard(AF.Ln)
        return tabs

    bacc.get_activation_tables = patched
    bacc._ant_expln_pinned = True


def _act_raw(nc, out, in_, func, bias=0.0, scale=1.0):
    """Emit InstActivation directly (used for Reciprocal, which the
    nc.scalar.activation wrapper refuses; measured ~1.2e-5 rel err)."""
    import concourse.mybir as mybir
    eng = nc.scalar
    inputs = [eng.lower_ap(in_)]
    for arg in (bias, scale, 0.0):
        inputs.append(mybir.ImmediateValue(dtype=mybir.dt.float32,
                                           value=float(arg)))
    return eng.add_instruction(mybir.InstActivation(
        name=nc.get_next_instruction_name(), func=func, ins=inputs,
        outs=[eng.lower_ap(out)]))


def _build_program(tiles, cls_idx, win_offs, Sw):
    import concourse.bacc as bacc
    import concourse.mybir as mybir
    from concourse.tile import TileContext
    from concourse.tile_rust import add_dep_helper
    from concourse.alu_op_type import AluOpType
    from concourse.dve_ops import RECIP_APPROX_FAST_CONSTS as RC
    import concourse.dve_ops as dve_ops

    f32 = mybir.dt.float32
    f16 = mybir.dt.float16
    f8 = mybir.dt.float8e4
    AF = mybir.ActivationFunctionType
    DR = mybir.MatmulPerfMode.DoubleRow
    taylor_den = _register_taylor_den16()
    recip_fast = next(o for o in dve_ops.OPS if o.name == "RECIPROCAL_APPROX_FAST")
    _pin_exp_ln_table()

    nc = bacc.Bacc()
    n_tiles = len(tiles)
    n_win_rows = int(win_offs[-1]) // 2
    xwin = nc.declare_dram_parameter("xwin", [2 * n_win_rows, _SLICE], f16,
                                     isOutput=False)
    wcpack_d = nc.declare_dram_parameter("wcpack", [128, 128 * n_tiles], f8,
                                         isOutput=False)
    w1pack_d = nc.declare_dram_parameter("w1pack", [45, 512], f16, isOutput=False)
    cst32_d = nc.declare_dram_parameter("cst32", [128, 45], f32, isOutput=False)
    fw2t_d = nc.declare_dram_parameter("fw2t", [128, 10], f16, isOutput=False)
    out_d = nc.declare_dram_parameter("out", [_PC, 10], f32, isOutput=True)

    zscale = 1.5 / (16.0 * Sw)

    with TileContext(nc) as tc:
        with (
            tc.tile_pool(name="const", bufs=1) as cpool,
            tc.tile_pool(name="xw", bufs=12) as xpool,
            tc.tile_pool(name="work", bufs=4) as wpool,
            tc.tile_pool(name="cps", bufs=3, space="PSUM") as cps,
            tc.tile_pool(name="zps", bufs=2, space="PSUM") as zps,
        ):
            # head consts load via the GpSimd queue so their issue overlaps
            # the Sync queue's first window DMAs
            w1pack_sb = cpool.tile_from(w1pack_d[:], name="w1pack_sb",
                                        forced_dma_engine=mybir.EngineType.Pool)
            cst32_sb = cpool.tile_from(cst32_d[:], name="cst32_sb",
                                       forced_dma_engine=mybir.EngineType.Pool)
            biaspack_sb = cst32_sb[:, 0:4]
            nbcomb_sb = cst32_sb[:, 4:5]
            fb2r_sb = cst32_sb[:, 5:45]
            # wcpack/fw2t SBUF space is allocated now but the (large) wcpack
            # DMA is deferred until after the first window loads, and fw2t's
            # until the tail, so the PE can start conv1 early.
            wcpack_sb = cpool.tile([128, 128 * n_tiles], f8, tag="wcpack",
                                   name="wcpack_sb", bufs=1)
            fw2t_sb = cpool.tile([128, 10], f16, tag="fw2t", name="fw2t_sb",
                                 bufs=1)

            # single-sync-wait rule: pre-observe PE-read const queues with
            # dummy 1-col matmuls; DVE/ACT-read consts with dummy touches.
            # The dummies write into a slice of the cp psum rotation.
            dps = cps.tile([128, 2 * _SLICE], f32, tag="cp", name="dps")
            nc.tensor.matmul(dps[0:128, 0:1], w1pack_sb[0:45, 0:128],
                             w1pack_sb[0:45, 0:1], start=True, stop=True)
            dvescr = wpool.tile([128, 45], f32, tag="dvescr", name="dvescr", bufs=1)
            nc.vector.tensor_copy(out=dvescr[:], in_=cst32_sb[:])
            actscr = wpool.tile([128, 1], f32, tag="actscr", name="actscr", bufs=1)
            nc.scalar.copy(out=actscr[:], in_=nbcomb_sb)

            zs = []
            # super-groups of 4 tiles (2 psum pairs); a trailing group of 2
            # when n_tiles % 4 == 2
            gbounds = list(range(0, n_tiles, 4))
            groups = [(b, min(4, n_tiles - b)) for b in gbounds]
            recip_insts = []

            def emit_dr(z, s, gbase, ntg):
                # fp8 DoubleRow matmuls for one group, contracting chunk
                # pairs 256-deep into z
                for dr in range(ntg // 2):
                    i0 = gbase + 2 * dr
                    lhsT = wcpack_sb[0:128, 128 * i0:128 * i0 + 256] \
                        .rearrange("p (i m) -> p i m", i=2)
                    rhs = s[:, dr * 2 * _SLICE:(dr + 1) * 2 * _SLICE] \
                        .rearrange("p (i n) -> p i n", i=2)
                    nc.tensor.matmul(z, lhsT, rhs,
                                     start=(i0 == 0),
                                     stop=(i0 + 2 == n_tiles),
                                     perf_mode=DR)

            # software pipelining: group g's DR matmuls are emitted after
            # group g+1's conv matmuls, so the PE never idles waiting for
            # the reciprocal's s-tile (the next convs fill that latency).
            pending_dr = None
            for sl in range(_NSL):
                z = zps.tile([128, _SLICE], f32, tag="z", name=f"z{sl}")
                zs.append(z)
                for sp, (gbase, ntg) in enumerate(groups):
                    tg = tiles[gbase:gbase + ntg]
                    q = wpool.tile([128, 4 * _SLICE], f32, tag="q",
                                   name=f"q{sl}_{sp}")
                    s = wpool.tile([128, 4 * _SLICE], f8, tag="s",
                                   name=f"s{sl}_{sp}")
                    for half in range(ntg // 2):   # one psum pair per half
                        # alternate the DMA-issue engine per psum pair: the
                        # Sync queue alone spends ~0.8us per dma_start and
                        # becomes a bottleneck; GpSimd is otherwise idle.
                        # Per-half (not per-group) alternation also issues the
                        # first group's windows on both queues in parallel,
                        # shortening the pipeline ramp.
                        dma_eng = nc.sync if (half % 2 == 0) else nc.gpsimd
                        cp = cps.tile([128, 2 * _SLICE], f32, tag="cp",
                                      name=f"cp{sl}_{sp}_{half}")
                        for j in range(2):
                            t = tg[2 * half + j]
                            i = gbase + 2 * half + j
                            xw = xpool.tile([t["K"], _SLICE], f16, tag="xw",
                                            name=f"xw{sl}_{i}")
                            ro = int(win_offs[i]) + sl * t["K"]
                            dma_eng.dma_start(out=xw, in_=xwin[ro:ro + t["K"], :])
                            if sl == 0 and sp == 0 and half == 0 and j == 1:
                                # big const load, after the first window DMAs
                                nc.sync.dma_start(out=wcpack_sb,
                                                  in_=wcpack_d[:])
                            if sl == 0 and sp == 2 and half == 0 and j == 0:
                                # tail-only const, loaded once the head clears
                                nc.sync.dma_start(out=fw2t_sb, in_=fw2t_d[:])
                            ci = cls_idx[t["cls"]]
                            nc.tensor.matmul(
                                cp[:, j * _SLICE:(j + 1) * _SLICE],
                                w1pack_sb[0:t["K"], 128 * ci:128 * ci + 128], xw,
                                start=True, stop=True)
                        ca = cls_idx[tg[2 * half]["cls"]]
                        cb = cls_idx[tg[2 * half + 1]["cls"]]
                        qh = q[:, half * 2 * _SLICE:(half + 1) * 2 * _SLICE]
                        if ca == cb:
                            nc.vector._custom_dve(
                                taylor_den, out=qh, in0=cp,
                                s0=biaspack_sb[0:128, ca:ca + 1], s1=2.0, imm2=3.0)
                        else:
                            nc.vector._custom_dve(
                                taylor_den, out=qh[:, 0:_SLICE],
                                in0=cp[:, 0:_SLICE],
                                s0=biaspack_sb[0:128, ca:ca + 1], s1=2.0, imm2=3.0)
                            nc.vector._custom_dve(
                                taylor_den, out=qh[:, _SLICE:2 * _SLICE],
                                in0=cp[:, _SLICE:2 * _SLICE],
                                s0=biaspack_sb[0:128, cb:cb + 1], s1=2.0, imm2=3.0)
                    if sl == 0 and sp == 0:
                        # pre-observe the wcpack PE-read queue only after the
                        # first conv matmuls, so the PE isn't head-blocked on
                        # the wcpack DMA.
                        nc.tensor.matmul(dps[0:128, 1:2],
                                         wcpack_sb[0:128, 0:128],
                                         wcpack_sb[0:128, 0:1],
                                         start=True, stop=True)
                    if sl == _NSL - 1 and sp == len(groups) - 1:
                        # the LAST reciprocal runs on the DVE (plain 1/den16;
                        # the missing 16x is folded into this group's wcpack
                        # columns) so the ScalarE queue reaches the natlog
                        # table load ~1.2us earlier and the tail exp chain
                        # starts while the final group still computes.
                        nc.vector._custom_dve(
                            recip_fast, out=s[:, 0:ntg * _SLICE],
                            in0=q[:, 0:ntg * _SLICE],
                            s0=RC["s0"], s1=RC["s1"], imm2=RC["imm2"])
                    else:
                        # s8 = 16/den16 in fp8e4m3 (range (0, ~8])
                        ri = _act_raw(nc, s[:, 0:ntg * _SLICE],
                                      q[:, 0:ntg * _SLICE],
                                      AF.Reciprocal, bias=0.0, scale=1.0 / 16.0)
                        recip_insts.append(ri)
                    if pending_dr is not None:
                        emit_dr(*pending_dr)
                    pending_dr = (z, s, gbase, ntg)
            emit_dr(*pending_dr)
            # pre-observe fw2t's PE-read queue before its first real use
            dps2 = cps.tile([128, 2 * _SLICE], f32, tag="cp", name="dps2")
            nc.tensor.matmul(dps2[0:10, 0:1], fw2t_sb[0:128, 0:10],
                             fw2t_sb[0:128, 0:1], start=True, stop=True)
            # ---- tail: sigmoid(fc1) via exp + DVE 1/(1+e), fc2, log_softmax
            # (no max-sub: |logits| < 12, exp cannot overflow fp32). All tail
            # ScalarE work is Exp/Ln (one natural_log_exp table load); order
            # it after the last reciprocal so the table sets load once each.
            last_recip = recip_insts[-1]
            hs = []
            for sl in range(_NSL):
                e1 = wpool.tile([128, _SLICE], f32, tag="e1", name=f"e1{sl}")
                ei = nc.scalar.activation(e1, zs[sl], AF.Exp,
                                          bias=nbcomb_sb[:], scale=-zscale)
                add_dep_helper(ei.ins, last_recip.ins, sync=False,
                               reason="keep tail ACT after recips (table sets)")
                ep = wpool.tile([128, _SLICE], f32, tag="ep", name=f"ep{sl}")
                # +1 on the DVE: the tail's critical engine is ScalarE (the
                # two exps), so the add must stay off its queue
                nc.vector.tensor_scalar(out=ep, in0=e1, scalar1=1.0,
                                        scalar2=None, op0=AluOpType.add)
                h = wpool.tile([128, _SLICE], f16, tag="h", name=f"h{sl}")
                nc.vector._custom_dve(recip_fast, out=h, in0=ep,
                                      s0=RC["s0"], s1=RC["s1"], imm2=RC["imm2"])
                hs.append(h)
            ng = _SLICE // 128
            ot = wpool.tile([128, _NSL * 10 * ng], f32, tag="ot", name="ot",
                            bufs=1)
            for sl in range(_NSL):
                fpt = zps.tile([128, _SLICE], f32, tag="z", name=f"fpt{sl}")
                fp = fpt[:, 0:10 * ng]
                for g in range(ng):
                    nc.tensor.matmul(fp[:, g * 10:(g + 1) * 10],
                                     hs[sl][:, g * 128:(g + 1) * 128], fw2t_sb[:],
                                     start=True, stop=True)
                lg = wpool.tile([128, 10 * ng], f32, tag="lg", name=f"lg{sl}")
                nc.vector.tensor_tensor(out=lg, in0=fp, in1=fb2r_sb[:, 0:10 * ng],
                                        op=AluOpType.add)
                e = wpool.tile([128, 10 * ng], f32, tag="e", name=f"e{sl}")
                ei = nc.scalar.activation(e, lg, AF.Exp)
                add_dep_helper(ei.ins, last_recip.ins, sync=False,
                               reason="keep tail ACT after recips (table sets)")
                ssum = wpool.tile([128, ng], f32, tag="ss", name=f"ss{sl}")
                nc.vector.tensor_reduce(
                    ssum, e.rearrange("p (g k) -> p g k", k=10),
                    axis=mybir.AxisListType.X, op=AluOpType.add)
                lns = wpool.tile([128, ng], f32, tag="ls", name=f"ls{sl}")
                li = nc.scalar.activation(lns, ssum, AF.Ln)
                add_dep_helper(li.ins, last_recip.ins, sync=False,
                               reason="keep tail ACT after recips (table sets)")
                for g in range(ng):
                    nc.vector.tensor_scalar(
                        out=ot[:, sl * 10 * ng + g * 10:sl * 10 * ng + (g + 1) * 10],
                        in0=lg[:, g * 10:(g + 1) * 10],
                        scalar1=lns[:, g:g + 1], scalar2=None,
                        op0=AluOpType.subtract)
            # one output DMA for the whole core: out rows = sl*512 + g*128 + p
            nc.sync.dma_start(
                out=out_d[:].rearrange("(s g p) k -> p s g k", p=128, s=_NSL),
                in_=ot.rearrange("p (s g k) -> p s g k", k=10, s=_NSL))
    nc.compile()
    return nc


_PROGRAM_CACHE = {}


def kernel(x, w1, b1, w2, b2, fw1, fb1, fw2, fb2):
    global LAST_RESULTS
    wins, consts, tiles = _host_prep(x, w1, b1, w2, b2, fw1, fb1, fw2, fb2)

    if "nc" not in _PROGRAM_CACHE:
        _PROGRAM_CACHE["nc"] = _build_program(tiles, consts["cls_idx"],
                                              consts["win_offs"], consts["Sw"])
    nc = _PROGRAM_CACHE["nc"]

    shared = {k: consts[k] for k in
              ("wcpack", "w1pack", "cst32", "fw2t")}
    in_maps = []
    for c in range(_NCORES):
        m = dict(shared)
        # per-core pre-windowed blob: per tile, per slice, [K_t, 512] blocks
        blocks = []
        for t_i, t in enumerate(tiles):
            w = wins[t_i][:, c * _PC:(c + 1) * _PC]
            for sl in range(_NSL):
                blocks.append(w[:, sl * _SLICE:(sl + 1) * _SLICE])
        m["xwin"] = np.ascontiguousarray(np.concatenate(blocks, axis=0))
        in_maps.append(m)

    from concourse.bass_utils import run_bass_kernel_spmd
    trace = bool(int(os.environ.get("BASS_KERNEL_TRACE", "0")))
    res = run_bass_kernel_spmd(nc, in_maps, core_ids=list(range(_NCORES)),
                               trace=trace)
    LAST_RESULTS = res
    return np.concatenate([r["out"] for r in res.results], axis=0)
